# revision 21
# baseline (speedup 1.0000x reference)
"""Trainium2 kernel for nn_ApproxMultLayer.

The reference quantizes x[32,256] and w[256,256] to uint8, applies an
approximate 8x8-bit multiplier circuit elementwise and reduces over the
inner dim: acc[b,o] = sum_i T[xq[b,i], wq[o,i]], out = acc / 255^2.

Structure of the circuit (verified exhaustively on all 2^16 pairs):

    T[a,b] = 256*F1[ah,bh] + 16*F[al,bh] + 16*F[ah,bl] + F[al,bl]

where F1/F are 16x16 nibble tables, the final ripple-add is exact, and
the sum never wraps 2^16.  Moreover F1[p,q] = p*q EXACTLY (the
high-nibble path is an exact multiplier), and F = p*q + R with the
approximation residual R in [-34, 0].  Hence

    T[a,b] = a*b + 16*R[al,bh] + 16*R[ah,bl] + R[al,bl]

so  acc = xq @ wq^T  (exact integer matmul, K=256)  plus three small
residual corrections.  Each residual term sum_i R[xnib[b,i], wnib[o,i]]
is a contraction through the 16x16 table R; we low-rank factor R = U V^T
(numerical rank 10, fast spectral decay) and contract

    sum_i sum_r U[xnib[b,i], r] * V[r, wnib[o,i]]

as extra K-rows of the same matmul.  With rA=3 (wh-group) and rB=4
(wl-group, two x-planes folded into the stationary M dim) the per-core
contraction is K = 32*(1 + rA + rB) = 256 = 2 k-tiles, and the
end-to-end max relative error is ~1e-3 (vs the 2e-2 gate); the dominant
xq@wq^T term is bit-exact (integers < 2^24 accumulated in fp32 PSUM).

Sharding: contraction split over the 8 cores (32 of the 256 i's each);
each core emits a [64, 256] fp32 partial (two M-plane halves), host
folds halves + sums cores.
"""

import numpy as np
import ml_dtypes


def _ensure_ntff_hook():
    """bass_utils imports antenv.axon_hooks when trace=True under axon;
    some images lack that module. Provide it (and register the ctypes
    hook the boot shim would have registered) so tracing works instead
    of crashing."""
    import importlib
    import sys
    import types

    try:
        hooks = importlib.import_module("antenv.axon_hooks")
    except ImportError:
        hooks = types.ModuleType("antenv.axon_hooks")
        hooks._axon_ntff_profile_hook = None

        def set_axon_ntff_profile_hook(h, _m=hooks):
            _m._axon_ntff_profile_hook = h

        def get_axon_ntff_profile_hook(_m=hooks):
            return _m._axon_ntff_profile_hook

        hooks.set_axon_ntff_profile_hook = set_axon_ntff_profile_hook
        hooks.get_axon_ntff_profile_hook = get_axon_ntff_profile_hook
        sys.modules["antenv.axon_hooks"] = hooks

    if hooks.get_axon_ntff_profile_hook() is None:
        try:
            from trn_agent_boot.trn_boot import _ntff_profile_via_ctypes

            hook = _ntff_profile_via_ctypes("/opt/axon/libaxon_pjrt.so")
            if hook is not None:
                hooks.set_axon_ntff_profile_hook(hook)
        except Exception:
            pass  # tracing degrades; compile + run still work


_ensure_ntff_hook()

SCALE = 255.0
B, IN, OUT = 32, 256, 256
N_CORES = 8
KPC = 128  # per-core contraction rows: [main 32 | t1 32 | t2 32 | t3 32]


# ---------------------------------------------------------------------------
# Approximate-multiplier residual table (numpy re-impl of the circuit)
# ---------------------------------------------------------------------------

def _badd4(a, b, c, d, cin):
    t = a + b + c + d + cin
    return t // 2, t % 2


def _grid4(Ab, Bb):
    G = [[0] * 8 for _ in range(4)]
    for r in range(4):
        for k in range(4):
            G[r][(4 - r) + k] = Ab[k] & Bb[3 - r]
    return G


def _reduce4(G):
    R = [0] * 8
    R[7] = G[0][7] | G[1][7] | G[2][7] | G[3][7]
    R[6] = G[0][6] | G[1][6] | G[2][6] | G[3][6]
    p1 = G[0][5] ^ G[1][5]
    p2 = G[2][5] ^ G[3][5]
    R[5] = p1 ^ p2
    carry = (p1 & p2) | (G[0][5] & G[1][5])
    R[4] = G[0][4] ^ G[1][4] ^ G[2][4] ^ G[3][4] ^ carry
    c = 0
    for col in (3, 2, 1, 0):
        c, R[col] = _badd4(G[0][col], G[1][col], G[2][col], G[3][col], c)
    return R


def _build_factors():
    n = np.arange(16, dtype=np.int64)
    x, y = n[:, None], n[None, :]
    xb = [(x >> (3 - i)) & 1 for i in range(4)]
    yb = [(y >> (3 - i)) & 1 for i in range(4)]
    Rr = _reduce4(_grid4(xb, yb))
    F = sum(Rr[i] << (7 - i) for i in range(8)).astype(np.float64)
    R = F - (x * y).astype(np.float64)  # approximation residual, in [-34, 0]

    bf16 = ml_dtypes.bfloat16

    def rank1(T):
        u, s, vt = np.linalg.svd(T, full_matrices=False)
        return (u[:, :1] * s[:1]).astype(bf16), vt[:1, :].astype(bf16)

    u1, v1 = rank1(16.0 * R)  # term1: x-nib=xl, w-nib=wh
    u2, v2 = rank1(16.0 * R)  # term2: x-nib=xh, w-nib=wl
    u3, v3 = rank1(1.0 * R)  # term3: x-nib=xl, w-nib=wl
    return (u1, v1), (u2, v2), (u3, v3)


_TERMS = _build_factors()


# ---------------------------------------------------------------------------
# Bass program (built once; same NEFF on all 8 cores)
# ---------------------------------------------------------------------------

_BASS_CACHE = {}


def _get_bass():
    if "nc" in _BASS_CACHE:
        return _BASS_CACHE["nc"]
    import concourse.bass as bass
    import concourse.mybir as mybir

    nc = bass.Bass()
    # fused input: per k-row, cols 0:32 = stationary planes, 32:288 = moving
    inp = nc.declare_dram_parameter(
        "inp", [128, B + OUT], mybir.dt.bfloat16, isOutput=False
    )
    out = nc.declare_dram_parameter(
        "out", [B, OUT], mybir.dt.float32, isOutput=True
    )

    # no nc.Block(): emit straight into the main BB — the per-engine
    # streams are ordered by the explicit semaphores alone, and the
    # block entry/exit all-engine barriers disappear.
    with (
        nc.sbuf_tensor([128, B + OUT], mybir.dt.bfloat16) as it,
        nc.sbuf_tensor([B, OUT], mybir.dt.float32) as osb,
        nc.psum_tensor([B, OUT], mybir.dt.float32) as psum,
        nc.semaphore("dsem") as dsem,
        nc.semaphore("psem") as psem,
        nc.semaphore("vsem") as vsem,
    ):
        # input on the ACT HWDGE ring — the scalar sequencer clears its
        # framework preamble earlier than SP, so the dispatch starts sooner
        nc.scalar.dma_start(it[:], inp[:]).then_inc(dsem, 16)
        nc.tensor.wait_ge(dsem, 16)
        nc.tensor.matmul(
            psum[:],
            lhsT=it[:, 0:B],
            rhs=it[:, B : B + OUT],
            start=True,
            stop=True,
        ).then_inc(psem, 1)
        nc.vector.wait_ge(psem, 1)
        nc.vector.tensor_copy(osb[:], psum[:]).then_inc(vsem, 1)
        nc.sync.wait_ge(vsem, 1)
        # out DMA completion is covered by the framework epilogue
        nc.sync.dma_start(out[:], osb[:]).then_inc(dsem, 16)

    _BASS_CACHE["nc"] = nc
    return nc


# ---------------------------------------------------------------------------
# Host-side prep + launch
# ---------------------------------------------------------------------------

last_results = None  # BassKernelResults of the most recent launch (for profiling)


def _quantize(v):
    # match jnp: f32 multiply, round-half-even, clip
    vq = np.clip(np.round(v.astype(np.float32) * np.float32(SCALE)), 0.0, 255.0)
    return vq.astype(np.int64)


def kernel(x, w):
    from concourse.bass_utils import run_bass_kernel_spmd

    x = np.asarray(x)
    w = np.asarray(w)
    xq = _quantize(x)  # [B, IN]
    wq = _quantize(w)  # [OUT, IN]
    xh, xl = xq >> 4, xq & 15
    wh, wl = wq >> 4, wq & 15

    bf16 = ml_dtypes.bfloat16
    f32 = np.float32
    (u1, v1), (u2, v2), (u3, v3) = [
        (u.astype(f32)[:, 0], v.astype(f32)[0, :]) for u, v in _TERMS
    ]

    # per-core 128 k-rows: [main(32) | t1(32) | t2(32) | t3(32)], M = 32 (batch)
    L = np.empty((N_CORES, 4, 32, B), dtype=f32)  # stationary
    Rm = np.empty((N_CORES, 4, 32, OUT), dtype=f32)  # moving

    def seg(xside, wside):  # [i, b], [i, o] -> per-core [c, 32, *]
        return (
            xside.reshape(N_CORES, 32, B),
            wside.reshape(N_CORES, 32, OUT),
        )

    L[:, 0], Rm[:, 0] = seg(xq.T.astype(f32), wq.T.astype(f32))
    L[:, 1], Rm[:, 1] = seg(u1[xl.T], v1[wh.T])
    L[:, 2], Rm[:, 2] = seg(u2[xh.T], v2[wl.T])
    L[:, 3], Rm[:, 3] = seg(u3[xl.T], v3[wl.T])

    full = np.concatenate(
        [L.reshape(N_CORES, KPC, B), Rm.reshape(N_CORES, KPC, OUT)], axis=2
    )  # [c, 128, 288]
    full = np.ascontiguousarray(full).astype(bf16)

    in_maps = [{"inp": full[c]} for c in range(N_CORES)]

    nc = _get_bass()
    res = run_bass_kernel_spmd(nc, in_maps, core_ids=list(range(N_CORES)))
    global last_results
    last_results = res

    acc = np.zeros((B, OUT), dtype=np.float64)
    for c in range(N_CORES):
        acc += res.results[c]["out"].astype(np.float64)  # [B, OUT]

    # match reference arithmetic: fp32 divide of the (near-integer) acc
    return acc.astype(np.float32) / np.float32(SCALE * SCALE)


# revision 22
# speedup vs baseline: 1.0062x; 1.0062x over previous
"""Trainium2 kernel for nn_ApproxMultLayer.

The reference quantizes x[32,256] and w[256,256] to uint8, applies an
approximate 8x8-bit multiplier circuit elementwise and reduces over the
inner dim: acc[b,o] = sum_i T[xq[b,i], wq[o,i]], out = acc / 255^2.

Structure of the circuit (verified exhaustively on all 2^16 pairs):

    T[a,b] = 256*F1[ah,bh] + 16*F[al,bh] + 16*F[ah,bl] + F[al,bl]

where F1/F are 16x16 nibble tables, the final ripple-add is exact, and
the sum never wraps 2^16.  Moreover F1[p,q] = p*q EXACTLY (the
high-nibble path is an exact multiplier), and F = p*q + R with the
approximation residual R in [-34, 0].  Hence

    T[a,b] = a*b + 16*R[al,bh] + 16*R[ah,bl] + R[al,bl]

so  acc = xq @ wq^T  (exact integer matmul, K=256)  plus three small
residual corrections.  Each residual term sum_i R[xnib[b,i], wnib[o,i]]
is a contraction through the 16x16 table R; we low-rank factor R = U V^T
(numerical rank 10, fast spectral decay) and contract

    sum_i sum_r U[xnib[b,i], r] * V[r, wnib[o,i]]

as extra K-rows of the same matmul.  With rA=3 (wh-group) and rB=4
(wl-group, two x-planes folded into the stationary M dim) the per-core
contraction is K = 32*(1 + rA + rB) = 256 = 2 k-tiles, and the
end-to-end max relative error is ~1e-3 (vs the 2e-2 gate); the dominant
xq@wq^T term is bit-exact (integers < 2^24 accumulated in fp32 PSUM).

Sharding: contraction split over the 8 cores (32 of the 256 i's each);
each core emits a [64, 256] fp32 partial (two M-plane halves), host
folds halves + sums cores.
"""

import numpy as np
import ml_dtypes


def _ensure_ntff_hook():
    """bass_utils imports antenv.axon_hooks when trace=True under axon;
    some images lack that module. Provide it (and register the ctypes
    hook the boot shim would have registered) so tracing works instead
    of crashing."""
    import importlib
    import sys
    import types

    try:
        hooks = importlib.import_module("antenv.axon_hooks")
    except ImportError:
        hooks = types.ModuleType("antenv.axon_hooks")
        hooks._axon_ntff_profile_hook = None

        def set_axon_ntff_profile_hook(h, _m=hooks):
            _m._axon_ntff_profile_hook = h

        def get_axon_ntff_profile_hook(_m=hooks):
            return _m._axon_ntff_profile_hook

        hooks.set_axon_ntff_profile_hook = set_axon_ntff_profile_hook
        hooks.get_axon_ntff_profile_hook = get_axon_ntff_profile_hook
        sys.modules["antenv.axon_hooks"] = hooks

    if hooks.get_axon_ntff_profile_hook() is None:
        try:
            from trn_agent_boot.trn_boot import _ntff_profile_via_ctypes

            hook = _ntff_profile_via_ctypes("/opt/axon/libaxon_pjrt.so")
            if hook is not None:
                hooks.set_axon_ntff_profile_hook(hook)
        except Exception:
            pass  # tracing degrades; compile + run still work


_ensure_ntff_hook()

SCALE = 255.0
B, IN, OUT = 32, 256, 256
N_CORES = 8
KPC = 128  # per-core contraction rows: [main 32 | t1 32 | t2 32 | t3 32]


# ---------------------------------------------------------------------------
# Approximate-multiplier residual table (numpy re-impl of the circuit)
# ---------------------------------------------------------------------------

def _badd4(a, b, c, d, cin):
    t = a + b + c + d + cin
    return t // 2, t % 2


def _grid4(Ab, Bb):
    G = [[0] * 8 for _ in range(4)]
    for r in range(4):
        for k in range(4):
            G[r][(4 - r) + k] = Ab[k] & Bb[3 - r]
    return G


def _reduce4(G):
    R = [0] * 8
    R[7] = G[0][7] | G[1][7] | G[2][7] | G[3][7]
    R[6] = G[0][6] | G[1][6] | G[2][6] | G[3][6]
    p1 = G[0][5] ^ G[1][5]
    p2 = G[2][5] ^ G[3][5]
    R[5] = p1 ^ p2
    carry = (p1 & p2) | (G[0][5] & G[1][5])
    R[4] = G[0][4] ^ G[1][4] ^ G[2][4] ^ G[3][4] ^ carry
    c = 0
    for col in (3, 2, 1, 0):
        c, R[col] = _badd4(G[0][col], G[1][col], G[2][col], G[3][col], c)
    return R


def _build_factors():
    n = np.arange(16, dtype=np.int64)
    x, y = n[:, None], n[None, :]
    xb = [(x >> (3 - i)) & 1 for i in range(4)]
    yb = [(y >> (3 - i)) & 1 for i in range(4)]
    Rr = _reduce4(_grid4(xb, yb))
    F = sum(Rr[i] << (7 - i) for i in range(8)).astype(np.float64)
    R = F - (x * y).astype(np.float64)  # approximation residual, in [-34, 0]

    bf16 = ml_dtypes.bfloat16

    def rank1(T):
        u, s, vt = np.linalg.svd(T, full_matrices=False)
        return (u[:, :1] * s[:1]).astype(bf16), vt[:1, :].astype(bf16)

    u1, v1 = rank1(16.0 * R)  # term1: x-nib=xl, w-nib=wh
    u2, v2 = rank1(16.0 * R)  # term2: x-nib=xh, w-nib=wl
    u3, v3 = rank1(1.0 * R)  # term3: x-nib=xl, w-nib=wl
    return (u1, v1), (u2, v2), (u3, v3)


_TERMS = _build_factors()


# ---------------------------------------------------------------------------
# Bass program (built once; same NEFF on all 8 cores)
# ---------------------------------------------------------------------------

_BASS_CACHE = {}


def _get_bass():
    if "nc" in _BASS_CACHE:
        return _BASS_CACHE["nc"]
    import concourse.bass as bass
    import concourse.mybir as mybir

    nc = bass.Bass(use_seq_codegen=True)
    # fused input: per k-row, cols 0:32 = stationary planes, 32:288 = moving
    inp = nc.declare_dram_parameter(
        "inp", [128, B + OUT], mybir.dt.bfloat16, isOutput=False
    )
    out = nc.declare_dram_parameter(
        "out", [B, OUT], mybir.dt.float32, isOutput=True
    )

    # no nc.Block(): emit straight into the main BB — the per-engine
    # streams are ordered by the explicit semaphores alone, and the
    # block entry/exit all-engine barriers disappear.
    with (
        nc.sbuf_tensor([128, B + OUT], mybir.dt.bfloat16) as it,
        nc.sbuf_tensor([B, OUT], mybir.dt.float32) as osb,
        nc.psum_tensor([B, OUT], mybir.dt.float32) as psum,
        nc.semaphore("dsem") as dsem,
        nc.semaphore("psem") as psem,
        nc.semaphore("vsem") as vsem,
    ):
        nc.sync.dma_start(it[:], inp[:]).then_inc(dsem, 16)
        nc.tensor.wait_ge(dsem, 16)
        nc.tensor.matmul(
            psum[:],
            lhsT=it[:, 0:B],
            rhs=it[:, B : B + OUT],
            start=True,
            stop=True,
        ).then_inc(psem, 1)
        nc.vector.wait_ge(psem, 1)
        nc.vector.tensor_copy(osb[:], psum[:]).then_inc(vsem, 1)
        nc.sync.wait_ge(vsem, 1)
        # out DMA completion is covered by the framework epilogue
        nc.sync.dma_start(out[:], osb[:]).then_inc(dsem, 16)

    _BASS_CACHE["nc"] = nc
    return nc


# ---------------------------------------------------------------------------
# Host-side prep + launch
# ---------------------------------------------------------------------------

last_results = None  # BassKernelResults of the most recent launch (for profiling)


def _quantize(v):
    # match jnp: f32 multiply, round-half-even, clip
    vq = np.clip(np.round(v.astype(np.float32) * np.float32(SCALE)), 0.0, 255.0)
    return vq.astype(np.int64)


def kernel(x, w):
    from concourse.bass_utils import run_bass_kernel_spmd

    x = np.asarray(x)
    w = np.asarray(w)
    xq = _quantize(x)  # [B, IN]
    wq = _quantize(w)  # [OUT, IN]
    xh, xl = xq >> 4, xq & 15
    wh, wl = wq >> 4, wq & 15

    bf16 = ml_dtypes.bfloat16
    f32 = np.float32
    (u1, v1), (u2, v2), (u3, v3) = [
        (u.astype(f32)[:, 0], v.astype(f32)[0, :]) for u, v in _TERMS
    ]

    # per-core 128 k-rows: [main(32) | t1(32) | t2(32) | t3(32)], M = 32 (batch)
    L = np.empty((N_CORES, 4, 32, B), dtype=f32)  # stationary
    Rm = np.empty((N_CORES, 4, 32, OUT), dtype=f32)  # moving

    def seg(xside, wside):  # [i, b], [i, o] -> per-core [c, 32, *]
        return (
            xside.reshape(N_CORES, 32, B),
            wside.reshape(N_CORES, 32, OUT),
        )

    L[:, 0], Rm[:, 0] = seg(xq.T.astype(f32), wq.T.astype(f32))
    L[:, 1], Rm[:, 1] = seg(u1[xl.T], v1[wh.T])
    L[:, 2], Rm[:, 2] = seg(u2[xh.T], v2[wl.T])
    L[:, 3], Rm[:, 3] = seg(u3[xl.T], v3[wl.T])

    full = np.concatenate(
        [L.reshape(N_CORES, KPC, B), Rm.reshape(N_CORES, KPC, OUT)], axis=2
    )  # [c, 128, 288]
    full = np.ascontiguousarray(full).astype(bf16)

    in_maps = [{"inp": full[c]} for c in range(N_CORES)]

    nc = _get_bass()
    res = run_bass_kernel_spmd(nc, in_maps, core_ids=list(range(N_CORES)))
    global last_results
    last_results = res

    acc = np.zeros((B, OUT), dtype=np.float64)
    for c in range(N_CORES):
        acc += res.results[c]["out"].astype(np.float64)  # [B, OUT]

    # match reference arithmetic: fp32 divide of the (near-integer) acc
    return acc.astype(np.float32) / np.float32(SCALE * SCALE)
